# revision 34
# baseline (speedup 1.0000x reference)
"""AntiBiasL1Loss (segment_reduce over 5 grades) on 8 TRN2 NeuronCores.

Algorithm
---------
reference:  seg = round(y_true) in [0,5);  e = |y_pred - y_true|
            sums[g]   = segment_sum(e, seg);  counts[g] = segment_sum(1, seg)
            out = mean_g(sums[g]/counts[g])  over present groups.

Device-side (per core, data-parallel shard of N/8 elements):
  d   = y_pred - y_true                  (DVE tensor_tensor, f32 -> bf16)
  e   = |d| = abs_max(d, 0)              (DVE tensor_scalar, bf16 4x)
  yb  = bf16(y_true)                     (DVE copy)
  m_t = (yb >= t-0.5), t = 1..4          (DVE tensor_scalar is_ge, bf16 4x)
  For each [128,128] chunk: 4 accumulating matmuls
      psum_t += e_chunk.T @ m_t_chunk    (TensorE, bf16, PSUM f32)

Sentinel trick: the host packs the data so that every 128-column chunk has
127 real columns plus one SENTINEL column (y_true=4, y_pred=5), i.e. e=1 and
all masks=1 for those elements.  In the accumulated [128,128] psum_t:
  diag[n,n], n<127   = masked segment sums  S_t = sum(e * [y >= t])
  row  [127, n<127]  = mask column-sums     C_t = count(y >= t)
  col  [m<127, 127]  = e column-sums        S_0 = sum(e)   (from psum_1)
so counts and sum(e) come out of the same 4 matmuls -- no accumulator
registers, no extra instructions.  Leftover capacity is padded with zero
columns (y=p=0): they add 0 to every sum and 0 to every mask/count.

Everything elementwise runs on the DVE and all DMA slots are single-use, so
every instruction carries at most ONE semaphore wait (walrus rejects more:
"Too many sync wait commands").  Input DMAs go through the gpsimd SWDGE
queue; the host interleaves y_pred|y_true per tile so each tile is one DMA.

Host-side finish: un-telescope sums[g] = S_g - S_{g+1}, counts likewise,
per-group means, final mean.  Output is a scalar, so no collective: each
core DMAs its 4 [128,128] psum blocks.
"""

import numpy as np

import concourse.bass as bass
from concourse import mybir, tile
from concourse import tile_sem_assignment as _tsa
from concourse.bass_utils import run_bass_kernel_spmd

# All SWDGE (gpsimd-issued) DMAs share one completion-sem lane: fewer DMA
# procs means fewer waits on the kernel-tail Drain, whose hardware encoding
# also has a small wait-slot budget.
_tsa.NUM_SWDGE_GLOBAL_SEMS = 1

P = 128
CORES = 8
N_TOTAL = 16_777_216
SHARD = N_TOTAL // CORES          # 2_097_152
FREE = SHARD // P                 # 16384 real columns per core
CHUNK = 128                       # matmul chunk cols (127 real + 1 sentinel)
REAL = CHUNK - 1
NCHUNK = -(-FREE // REAL)         # 130 chunks
NT = 5                            # tiles per core
TILE_C = NCHUNK // NT * CHUNK     # 3328 cols per tile (26 chunks)
TOTC = NT * TILE_C                # 16640 packed cols per core
THRESHOLDS = (0.5, 1.5, 2.5, 3.5)
F32 = mybir.dt.float32
BF16 = mybir.dt.bfloat16
assert NCHUNK % NT == 0 and NCHUNK * REAL >= FREE


def build_kernel(nt: int = NT, tile_c: int = TILE_C) -> bass.Bass:
    nch = tile_c // CHUNK
    nc = bass.Bass(target_bir_lowering=False, debug=False)

    # interleaved input: per tile j, cols [j*2T, j*2T+T) = y_pred,
    # [j*2T+T, (j+1)*2T) = y_true   (T = tile_c packed columns).  bf16: the
    # device rounds to bf16 anyway, so converting on the host halves DMA
    # traffic and doubles the DVE perf-mode rates.
    x_ext = nc.declare_dram_parameter("xin", [P, 2 * nt * tile_c], BF16, isOutput=False)
    out_ext = nc.declare_dram_parameter("out", [P, 4 * CHUNK], F32, isOutput=True)

    with tile.TileContext(nc) as tc:
        with (
            # bufs=nt: every input tile gets its own SBUF slot, so input DMAs
            # never carry a WAW wait from slot reuse (DMACopy encodes at most
            # one sync wait).
            tc.tile_pool(name="inp", bufs=nt) as inp,
            tc.tile_pool(name="mid", bufs=2) as mid,
            tc.tile_pool(name="stat", bufs=1) as stat,
            tc.tile_pool(name="psum", bufs=1, space=bass.MemorySpace.PSUM) as psum,
        ):
            psum_t = [
                psum.tile([P, CHUNK], F32, tag=f"ps{t}", name=f"ps{t}")
                for t in range(4)
            ]

            for j in range(nt):
                xt = inp.tile([P, 2 * tile_c], BF16, tag="xt")
                nc.gpsimd.dma_start(
                    out=xt[:, :], in_=x_ext[:, j * 2 * tile_c : (j + 1) * 2 * tile_c]
                )
                pt = xt[:, :tile_c]
                yt = xt[:, tile_c:]

                d = mid.tile([P, tile_c], BF16, tag="d")
                nc.vector.tensor_tensor(d[:, :], pt, yt, mybir.AluOpType.subtract)

                # Each op encodes at most ONE semaphore wait.  Every mask
                # reads y straight from the DMA tile (that RAW tick is
                # already observed via the subtract), so each op's single
                # wait slot takes the WAR wait on its own slot's previous
                # PSUM-group readers -- robust to matmul reordering.
                masks = []
                for t, thr in enumerate(THRESHOLDS):
                    m = mid.tile([P, tile_c], BF16, tag=f"m{t}")
                    nc.vector.tensor_scalar(
                        m[:, :], yt, thr, None, mybir.AluOpType.is_ge
                    )
                    masks.append(m)
                # |d| = max(-d, d) in ONE scalar_tensor_tensor op (the real
                # ISA's tensor_scalar has no abs-class ALU op).
                e = mid.tile([P, tile_c], BF16, tag="e")
                nc.vector.scalar_tensor_tensor(
                    e[:, :], d[:, :], -1.0, d[:, :],
                    mybir.AluOpType.mult, mybir.AluOpType.max,
                )

                for c in range(nch):
                    csl = slice(c * CHUNK, (c + 1) * CHUNK)
                    first = j == 0 and c == 0
                    last = j == nt - 1 and c == nch - 1
                    for t in range(4):
                        nc.tensor.matmul(
                            psum_t[t][:, :],
                            e[:, csl],
                            masks[t][:, csl],
                            start=first,
                            stop=last,
                        )

            psum_sb = stat.tile([P, 4 * CHUNK], F32, tag="psb", name="psum_sb")
            for t in range(4):
                nc.vector.tensor_copy(
                    psum_sb[:, t * CHUNK : (t + 1) * CHUNK], psum_t[t][:, :]
                )
            nc.sync.dma_start(out=out_ext[:, :], in_=psum_sb[:, :])

    # The kernel-tail Drain waits on every active proc, but its hardware
    # encoding (like every other instruction here) holds only ONE sync wait.
    # All but the final output DMA's completion are transitively implied:
    # engine queues are in-order and the all-engine barrier follows the
    # drain; every input-DMA completion was already awaited by its DVE
    # consumer.  Keep only the DMAHW wait (the output DMA).
    for b in nc.m.functions[0].blocks:
        for i in b.instructions:
            si = i.sync_info
            if type(i).__name__ == "InstDrain" and si and len(si.on_wait) > 1:
                keep = [w for w in si.on_wait if w.ant_name.startswith("DMAHW")]
                assert len(keep) == 1, [w.ant_name for w in si.on_wait]
                i.sync_info = mybir.SyncInfo(
                    on_wait=keep, on_update=list(si.on_update)
                )
    return nc


def combine_outputs(outs, n_total: int = N_TOTAL) -> np.float32:
    """Host-side finish: un-telescope sums/counts, per-group means, mean."""
    s_thr = np.zeros(4, np.float64)  # S_t = sum(e * [y >= t]), t=1..4
    c_thr = np.zeros(4, np.float64)  # C_t = count(y >= t)
    sum_e = 0.0
    for o in outs:
        o = np.asarray(o, np.float64)
        for t in range(4):
            blk = o[:, t * CHUNK : (t + 1) * CHUNK]
            s_thr[t] += np.trace(blk[:REAL, :REAL])
            c_thr[t] += blk[REAL, :REAL].sum()
        sum_e += o[:, 0 * CHUNK : 1 * CHUNK][:REAL, REAL].sum()

    s_cum = np.array([sum_e, *s_thr, 0.0])
    c_cum = np.array([float(n_total), *c_thr, 0.0])
    sums = s_cum[:-1] - s_cum[1:]
    counts = c_cum[:-1] - c_cum[1:]
    present = counts > 0
    means = np.where(present, sums / np.where(present, counts, 1.0), 0.0)
    return np.float32(means.sum() / present.sum())


def pack_inputs(y_pred: np.ndarray, y_true: np.ndarray):
    """[N] f32 x2 -> per-core bf16 [P, 2*TOTC]: sentinel col per chunk,
    zero-col padding, then per-tile y_pred|y_true interleave."""
    import ml_dtypes
    bf16 = np.dtype(ml_dtypes.bfloat16)
    p = np.ascontiguousarray(y_pred, np.float32).reshape(CORES, P, FREE)
    y = np.ascontiguousarray(y_true, np.float32).reshape(CORES, P, FREE)
    pc = np.zeros((CORES, P, NCHUNK, CHUNK), bf16)
    yc = np.zeros((CORES, P, NCHUNK, CHUNK), bf16)
    tmp = np.zeros((CORES, P, NCHUNK * REAL), np.float32)
    tmp[:, :, :FREE] = p
    pc[:, :, :, :REAL] = tmp.reshape(CORES, P, NCHUNK, REAL)
    tmp[:, :, :FREE] = y
    yc[:, :, :, :REAL] = tmp.reshape(CORES, P, NCHUNK, REAL)
    pc[:, :, :, REAL] = 5.0  # sentinel: e = |5-4| = 1
    yc[:, :, :, REAL] = 4.0  # sentinel: all masks = 1
    pc = pc.reshape(CORES, P, NT, TILE_C)
    yc = yc.reshape(CORES, P, NT, TILE_C)
    x = np.empty((CORES, P, NT, 2, TILE_C), bf16)
    x[:, :, :, 0, :] = pc
    x[:, :, :, 1, :] = yc
    return x.reshape(CORES, P, 2 * TOTC)


def run(y_pred: np.ndarray, y_true: np.ndarray, trace: bool = False, **kw):
    x = pack_inputs(y_pred, y_true)
    in_maps = [{"xin": x[i]} for i in range(CORES)]
    nc = build_kernel()
    res = run_bass_kernel_spmd(
        nc, in_maps, core_ids=list(range(CORES)), trace=trace, **kw
    )
    outs = [res.results[i]["out"] for i in range(CORES)]
    return np.asarray(combine_outputs(outs), np.float32), res


def kernel(y_pred: np.ndarray, y_true: np.ndarray) -> np.ndarray:
    return run(y_pred, y_true)[0]
